# revision 24
# baseline (speedup 1.0000x reference)
"""Trainium2 Bass kernel for DCTTransform (2D DCT -> 4 freq masks -> IDCT), v5.

Data parallel: 96 images of 512x512 across 8 cores (12 each).

v5 = v4's matmul structure (A/B split: E/O stage-1 partials fed through M4
separately, host does the +- recombination) with two regressions fixed:

* ALL input folds move to the HOST.  Both the row-fold (xa +- xr) and the
  column-fold (c' vs 511-c') commute with M1's row contraction, so the host
  ships xq = [xpp xpm xmp xmm] (each [128, 2, 256]) and M1 produces m1p/m1m
  DIRECTLY in PSUM.  Zero DVE/GpSimd input work on device, no m1n/m1r
  round-trip, and image 0's first matmul gates only on its input DMA.
* Two separate PSUM pools again (psA: M1/M2, psB: M3/M4), each a ring of
  [128, 1024] fp32 2-bank tiles with bufs=2 -- no cross-stage false
  dependencies (v4's single ring made M3(i) wait on M1(i+1)'s evacuation,
  stalling the PE into HAM re-throttle).

Evacuations are merged per 2-bank pair, balanced: scalar ~7.6us/img (m1p, y,
lh3, hl3, m4lh, m4hl), DVE ~7.5us/img (m1m, A1-TT, hh3, m4hh, masks).
GpSimd only does the B1 mask multiplies (v4's big GpSimd ops degraded DVE's
2x mode via SBUF port contention).

mask3 == ones -> LL == x, returned on host.  Outputs fp16, assembled on host.
"""

import sys

if "/opt/trn_rl_repo" not in sys.path:
    sys.path.insert(0, "/opt/trn_rl_repo")

import numpy as np

NCORES = 8
IMG = 512
P = 128
H = 256


def build_program(nimg):
    import concourse.bacc as bacc
    import concourse.tile as tile
    import concourse.mybir as mybir

    f32, f16 = mybir.dt.float32, mybir.dt.float16

    nc = bacc.Bacc("TRN2", target_bir_lowering=False, debug=False, num_devices=NCORES)

    # xq: host-prefolded inputs [t(pp,pm,mp,mm), k(row block), c' 256]
    xq_d = nc.dram_tensor("xq", [nimg, P, 4, 2, H], f16, kind="ExternalInput")
    CW0 = 2 * 512
    CWB = 2 * 512 + 512 + 256 + 384 + 192  # de do dsnr dlx trix t64
    cst0_d = nc.dram_tensor("cst0", [P, CW0], f16, kind="ExternalInput")
    cstB_d = nc.dram_tensor("cstB", [P, CWB], f16, kind="ExternalInput")
    a1m_d = nc.dram_tensor("a1m", [P, 512], f32, kind="ExternalInput")
    # merged output [nimg, P, 2(m2), 3(mask lh/hl/hh), 1024]:
    #   lh:  1024 = (A=even-f2 512 | B=odd-f2 512); host: out_n=A+B, out_r=A-B
    #   hl/hh: 1024 = (EA | EB | OA | OB) x 256  (A/B quadrants)
    out_d = nc.dram_tensor("out", [nimg, P, 2, 3, 1024], f16,
                           kind="ExternalOutput")

    def eo(ap3, lo, hi):
        return ap3.rearrange("p (g c) -> p g c", g=2)[:, :, lo:hi]

    with tile.TileContext(nc) as tc:
        with (
            tc.tile_pool(name="const", bufs=1) as cpool,
            tc.tile_pool(name="io", bufs=4) as iopool,
            tc.tile_pool(name="work", bufs=2) as wpool,
            tc.tile_pool(name="out", bufs=2) as opool,
            tc.tile_pool(name="psA", bufs=2, space="PSUM") as psA,
            tc.tile_pool(name="psB", bufs=4, space="PSUM") as psB,
        ):
            cst0 = cpool.tile([P, CW0], f16, tag="cst0")
            cstB = cpool.tile([P, CWB], f16, tag="cstB")
            a1m = cpool.tile([P, 512], f32, tag="a1m")

            ce = cst0[:, 0:512].rearrange("p (k h) -> p k h", k=2)
            co = cst0[:, 512:1024].rearrange("p (k h) -> p k h", k=2)

            def bview(lo, w, k=None):
                v = cstB[:, lo : lo + w]
                return v.rearrange("p (k h) -> p k h", k=k) if k else v

            de = bview(0, 512, 2)
            do = bview(512, 512, 2)
            dsnr = bview(1024, 512)
            dlx = bview(1536, 256)   # rows 0:64 = D[2b, s2q]; 64:128 = D[2b+1, s2q]
            trix = bview(1792, 384)
            t64 = cstB[0:64, 2176 : 2176 + 192]

            tri2 = eo(trix[:, 0:256], 0, 128)      # (tri | tri)   [p,2,128]
            trip2 = eo(trix[:, 128:384], 0, 128)   # (tri | trip)
            t642 = eo(t64[:, 0:128], 0, 64)        # (t64 | t64)   [64p,2,64]
            t64p2 = eo(t64[:, 64:192], 0, 64)      # (t64 | t64p)

            def stage_in(img, first=False):
                xq = iopool.tile([P, 4, 2, H], f16, tag="xq", name="xq")
                if first:
                    # split image-0's transfer so M1's m1p group can start
                    # after half the data; consts go via the scalar queue so
                    # their transfer overlaps xq0's instead of serializing
                    nc.sync.dma_start(xq[:, 0:2], xq_d[img, :, 0:2])
                    nc.scalar.dma_start(cst0[:], cst0_d[:])
                    nc.sync.dma_start(xq[:, 2:4], xq_d[img, :, 2:4])
                else:
                    nc.sync.dma_start(xq[:], xq_d[img])
                return xq

            def stage_front(img, xq):
                # M1: m1p/m1m directly (host did both folds).
                # layout [mp(c-chunk) 512 | ...], each (e 256 | o 256)
                m1pP = psA.tile([P, 1024], f32, tag="qa", name=f"m1pP{img}")
                m1mP = psA.tile([P, 1024], f32, tag="qa", name=f"m1mP{img}")
                for t, te, to in ((m1pP, 0, 1), (m1mP, 2, 3)):
                    for mp in range(2):
                        for par, (src, rhs) in enumerate(((te, ce), (to, co))):
                            s = 512 * mp + 256 * par
                            for k in range(2):
                                nc.tensor.matmul(
                                    t[:, s : s + 256],
                                    xq[:, src, k, P * mp : P * (mp + 1)],
                                    rhs[:, k, :], start=(k == 0), stop=(k == 1))
                m1p = wpool.tile([P, 2, IMG], f16, tag="m1p", name="m1p")
                m1m = wpool.tile([P, 2, IMG], f16, tag="m1m", name="m1m")
                nc.scalar.copy(m1p[:], m1pP[:].rearrange("p (g c) -> p g c", g=2))
                nc.vector.tensor_copy(m1m[:], m1mP[:].rearrange("p (g c) -> p g c", g=2))
                return m1p, m1m

            # HAM warmup: dummy matmuls on a memset scratch tile run during
            # the input-DMA wait, so the PE clock-gate is already released
            # (2.4 GHz) when the first real matmul issues.
            warm = wpool.tile([P, 256], f16, tag="warm", name="warm")
            nc.gpsimd.memset(warm[:], 0)
            wq = psB.tile([P, 512], f32, tag="qb", name="warmq")
            for _ in range(16):
                nc.tensor.matmul(wq[:, 0:256], warm[:, 0:128], warm[:, :],
                                 start=True, stop=True)

            ins = {0: stage_in(0, first=True)}
            nc.sync.dma_start(cstB[:], cstB_d[:])
            if nimg > 1:
                ins[1] = stage_in(1)
            nc.sync.dma_start(a1m[:], a1m_d[:])
            front = stage_front(0, ins.pop(0))
            # fill image-0's pipeline-fill gap (M2(0) waits m1p(0) evac)
            for _ in range(8):
                nc.tensor.matmul(wq[:, 0:256], warm[:, 0:128], warm[:, :],
                                 start=True, stop=True)
            for img in range(nimg):
                m1p, m1m = front
                if img + 2 < nimg:
                    ins[img + 2] = stage_in(img + 2)

                # ---- M2: y [128, 2(F: e0,o0), 512(f2: e|o)]; e1/o1 -> a1p,
                # masked straight into tmA1.
                y = wpool.tile([P, 2, IMG], f16, tag="y")
                FCR = [0, 256, 128, 384]
                yp = psA.tile([P, 1024], f32, tag="qa", name=f"yp{img}")
                a1p = psA.tile([P, 1024], f32, tag="qa", name=f"a1p{img}")
                for par, (m1, rhs) in enumerate(((m1p, ce), (m1m, co))):
                    for j in range(2):
                        c0 = FCR[j]
                        s = 512 * j + 256 * par
                        for k in range(2):
                            nc.tensor.matmul(
                                yp[:, s : s + 256], m1[:, k, c0 : c0 + P],
                                rhs[:, k, :], start=(k == 0), stop=(k == 1))
                # a1p: only f2 b<128 survives the HH mask -> N=128 matmuls,
                # packed [j, par, c<128] in bank 0
                for par, (m1, rhs) in enumerate(((m1p, ce), (m1m, co))):
                    for j in range(2):
                        c0 = FCR[2 + j]
                        s = 256 * j + 128 * par
                        for k in range(2):
                            nc.tensor.matmul(
                                a1p[:, s : s + 128], m1[:, k, c0 : c0 + P],
                                rhs[:, k, 0:128], start=(k == 0), stop=(k == 1))
                nc.scalar.copy(y[:], yp[:].rearrange("p (g c) -> p g c", g=2))
                # merged A1 mask multiply: [j(e1,o1), g(f2-parity), c<128]
                tmA1 = wpool.tile([P, 512], f16, tag="tmA1", name="tmA1")
                nc.vector.tensor_mul(
                    tmA1[:].rearrange("p (j g c) -> p j g c", j=2, g=2),
                    a1p[:, 0:512].rearrange("p (j g c) -> p j g c", j=2, g=2),
                    a1m[:].rearrange("p (j g c) -> p j g c", j=2, g=2))

                # image 0 only: M1(1) waits on yp(0)'s psA buffer; fill the
                # wait with dummies
                if img == 0:
                    for _ in range(6):
                        nc.tensor.matmul(wq[:, 0:256], warm[:, 0:128],
                                         warm[:, :], start=True, stop=True)

                # next image's M1 here: its PE burst covers the y-copy /
                # mask-multiply latency gap between M2 and M3 of this image,
                # and its psA allocs reuse yp/a1p only one image later.
                if img + 1 < nimg:
                    front = stage_front(img + 1, ins.pop(img + 1))

                # ---- masked tiles (fp16); all four big mask-muls on GpSimd
                # (it is otherwise idle; keeps DVE free for evacuations)
                tmLH = wpool.tile([P, P], f16, tag="tmLH")
                tmLH_o = wpool.tile([64, P], f16, tag="tmLHo")
                nc.vector.tensor_mul(tmLH[0:64, :].rearrange("p (g c) -> p g c", g=2),
                                     eo(y[0:64, 0, :], 0, 64), t642)
                nc.vector.tensor_mul(tmLH_o[:].rearrange("p (g c) -> p g c", g=2),
                                     eo(y[0:64, 1, :], 0, 64), t64p2)
                nc.sync.dma_start(tmLH[64:128, :], tmLH_o[:])
                tms = {}
                for nm, blk, lo, msk in (
                    ("HLe", 0, 0, tri2), ("HLo", 1, 0, trip2),
                    ("B1e", 0, 128, tri2), ("B1o", 1, 128, trip2),
                ):
                    t = wpool.tile([P, 256], f16, tag=f"tm{nm}")
                    nc.gpsimd.tensor_mul(t[:].rearrange("p (g c) -> p g c", g=2),
                                         eo(y[:, blk, :], lo, lo + 128), msk)
                    tms[nm] = t

                # ---- M3 (stage-1 inverse), E/O split, s1 < 256 only.
                # 1-bank psB tiles (ring-4) with per-bank evacs on alternating
                # engines keep the PE fed.
                vLH = wpool.tile([P, IMG], f16, tag="vLH")
                vhl = wpool.tile([P, 1024], f16, tag="vhl")
                vhh0 = wpool.tile([P, 1024], f16, tag="vhh0")
                vhh1 = wpool.tile([P, 1024], f16, tag="vhh1")

                def qb(nm_):
                    return psB.tile([P, 512], f32, tag="qb", name=f"{nm_}_{img}")

                # hhq0 banks: [E0(g) | O0(g)] for g = 0, 1 (2-acc groups)
                for g in range(2):
                    t = qb(f"hhq0{g}")
                    nc.tensor.matmul(t[:, 0:256], y[:, 0, 256 * g : 256 * g + P],
                                     de[:, 0, :], start=True, stop=False)
                    nc.tensor.matmul(t[:, 0:256], tmA1[:, 128 * g : 128 * g + P],
                                     de[:, 1, :], start=False, stop=True)
                    nc.tensor.matmul(t[:, 256:512], y[:, 1, 256 * g : 256 * g + P],
                                     do[:, 0, :], start=True, stop=False)
                    nc.tensor.matmul(t[:, 256:512], tmA1[:, 256 + 128 * g : 256 + 128 * g + P],
                                     do[:, 1, :], start=False, stop=True)
                    nc.vector.tensor_copy(vhh0[:, 512 * g : 512 * g + 512], t[:])

                lh3 = qb("lh3")
                nc.tensor.matmul(lh3[:, :], tmLH[:, :], dsnr, start=True, stop=True)
                nc.scalar.copy(vLH[:], lh3[:])

                # hl3 banks: [E(g) | O(g)], single matmuls
                for g in range(2):
                    t = qb(f"hl3{g}")
                    nc.tensor.matmul(t[:, 0:256], tms["HLe"][:, P * g : P * (g + 1)],
                                     de[:, 0, :], start=True, stop=True)
                    nc.tensor.matmul(t[:, 256:512], tms["HLo"][:, P * g : P * (g + 1)],
                                     do[:, 0, :], start=True, stop=True)
                    nc.scalar.copy(vhl[:, 512 * g : 512 * g + 512], t[:])

                # hhq1 banks: [E1(g) | O1(g)], single matmuls
                for g in range(2):
                    t = qb(f"hhq1{g}")
                    nc.tensor.matmul(t[:, 0:256], tms["B1e"][:, P * g : P * (g + 1)],
                                     de[:, 0, :], start=True, stop=True)
                    nc.tensor.matmul(t[:, 256:512], tms["B1o"][:, P * g : P * (g + 1)],
                                     do[:, 0, :], start=True, stop=True)
                    nc.vector.tensor_copy(vhh1[:, 512 * g : 512 * g + 512], t[:])

                # ---- M4 (stage-2 inverse), D-stationary; quads interleaved
                # across masks so scalar/DVE evacuations alternate.
                ot = opool.tile([P, 2, 3, 1024], f16, tag="ot", name="ot")
                MIDX = {"lh": 0, "hl": 1, "hh": 2}

                def m4quad(nm, m2):
                    c0 = P * m2
                    mi = MIDX[nm]
                    qa_ = qb(f"m4{nm}{m2}a")
                    qb_ = qb(f"m4{nm}{m2}b")
                    if nm == "lh":
                        # stacked dsn/dsr: qa = out_n, qb = out_r directly
                        nc.tensor.matmul(qa_[:, :], dsnr[:, c0 : c0 + P],
                                         vLH[:, :], start=True, stop=True)
                        nc.tensor.matmul(qb_[:, :], dsnr[:, 256 + c0 : 256 + c0 + P],
                                         vLH[:, :], start=True, stop=True)
                    else:
                        # [EA|EB] and [OA|OB] share a stationary operand and
                        # have adjacent rhs slices -> one N=512 matmul each
                        for t, dd, v0 in ((qa_, de, 0), (qb_, do, 512)):
                            if nm == "hl":
                                nc.tensor.matmul(t[:, :], dd[:, 0, c0 : c0 + P],
                                                 vhl[:, v0 : v0 + 512],
                                                 start=True, stop=True)
                            else:
                                nc.tensor.matmul(t[:, :], dd[:, 0, c0 : c0 + P],
                                                 vhh0[:, v0 : v0 + 512],
                                                 start=True, stop=False)
                                nc.tensor.matmul(t[:, :], dd[:, 1, c0 : c0 + P],
                                                 vhh1[:, v0 : v0 + 512],
                                                 start=False, stop=True)
                    # evac: first bank scalar, second DVE (except hl m2=1 on
                    # non-final images: both scalar, to balance totals)
                    nc.scalar.copy(ot[:, m2, mi, 0:512], qa_[:])
                    if nm == "hl" and m2 == 1 and img < nimg - 1:
                        nc.scalar.copy(ot[:, m2, mi, 512:1024], qb_[:])
                    else:
                        nc.vector.tensor_copy(ot[:, m2, mi, 512:1024], qb_[:])

                if img == nimg - 1:
                    # tail: ship each mask slice as soon as it is evacuated
                    for nm, m2 in (("hh", 0), ("hl", 0), ("lh", 0),
                                   ("hh", 1), ("hl", 1), ("lh", 1)):
                        m4quad(nm, m2)
                        nc.sync.dma_start(out_d[img, :, m2, MIDX[nm]],
                                          ot[:, m2, MIDX[nm]])
                else:
                    for nm, m2 in (("lh", 0), ("hh", 0), ("hl", 0),
                                   ("hh", 1), ("lh", 1), ("hl", 1)):
                        m4quad(nm, m2)
                    nc.sync.dma_start(out_d[img], ot[:])

    nc.compile()
    return nc


_prog_cache = {}

TRACE = False
TRACE_KWARGS = {}
LAST_RESULTS = None


def _get_prog(nimg):
    if nimg not in _prog_cache:
        _prog_cache[nimg] = build_program(nimg)
    return _prog_cache[nimg]


def _dct_f64():
    k = np.arange(IMG, dtype=np.float64)[:, None]
    m = np.arange(IMG, dtype=np.float64)[None, :]
    D = np.cos(np.pi * (2.0 * m + 1.0) * k / (2.0 * IMG)) * np.sqrt(2.0 / IMG)
    D[0] *= 1.0 / np.sqrt(2.0)
    return D


def _consts():
    D = _dct_f64()
    ce = D[0::2, 0:H].T.reshape(2, P, H).transpose(1, 0, 2)
    co = D[1::2, 0:H].T.reshape(2, P, H).transpose(1, 0, 2)
    de = D[0::2, 0:H].reshape(2, P, H).transpose(1, 0, 2)
    do = D[1::2, 0:H].reshape(2, P, H).transpose(1, 0, 2)
    ii = np.arange(P)[:, None]
    jj = np.arange(P)[None, :]
    tri = (ii + jj <= 127).astype(np.float64)
    trip = (ii + jj <= 126).astype(np.float64)
    i4 = np.arange(64)[:, None]
    j4 = np.arange(64)[None, :]
    t64 = (i4 + j4 <= 63).astype(np.float64)
    t64p = (i4 + j4 <= 62).astype(np.float64)
    dsn = np.concatenate([D[0:128:2, 0:H], D[1:128:2, 0:H]], 0)
    dsr = np.concatenate([D[0:128:2, 0:H], -D[1:128:2, 0:H]], 0)
    dsnr = np.concatenate([dsn, dsr], 1)
    dlx = np.concatenate([D[0:128:2, 0:H], D[1:128:2, 0:H]], 0)  # [e64; o64]
    trix = np.concatenate([tri, tri, trip], 1)
    t64w = np.zeros((P, 192))
    t64w[0:64] = np.concatenate([t64, t64, t64p], 1)
    cst0 = np.concatenate([ce.reshape(P, 512), co.reshape(P, 512)], axis=1)
    cstB = np.concatenate([
        de.reshape(P, 512), do.reshape(P, 512), dsnr, dlx, trix, t64w,
    ], axis=1)
    a1m = np.concatenate([tri, tri, tri, trip], axis=1)
    return {
        "cst0": np.ascontiguousarray(cst0).astype(np.float16),
        "cstB": np.ascontiguousarray(cstB).astype(np.float16),
        "a1m": np.ascontiguousarray(a1m).astype(np.float32),
    }


def _to_s1(t):
    return np.concatenate([t[:, :, 0:256], t[:, :, 256:512][:, :, ::-1]], 2)


def _assemble_lh(arr):
    """arr [n, 128, 2(m2), 1024=(A even-f2 512 | B odd-f2 512)] -> [n,512,512].

    out_n (normal s2 rows) = A + B; out_r (reflected) = A - B.
    """
    a = arr.astype(np.float32).reshape(-1, P, 2, 2, IMG)
    A = a[:, :, :, 0, :].transpose(0, 2, 1, 3).reshape(-1, 256, 512)
    B = a[:, :, :, 1, :].transpose(0, 2, 1, 3).reshape(-1, 256, 512)
    n = a.shape[0]
    out = np.empty((n, IMG, IMG), dtype=np.float32)
    out[:, :, 0:256] = _to_s1(A).transpose(0, 2, 1)
    out[:, :, 256:512] = _to_s1(B).transpose(0, 2, 1)[:, :, ::-1]
    return out


def _assemble_ab(arr):
    """arr [n, 128, 2(m2), 1024=(EA|EB|OA|OB)x256] -> [n, 512, 512]."""
    a = arr.astype(np.float32).reshape(-1, P, 2, 4, 256)
    c = a.transpose(0, 2, 1, 3, 4).reshape(-1, 256, 4, 256)  # [n, s2q, comp, s1q]
    EA, EB, OA, OB = c[:, :, 0], c[:, :, 1], c[:, :, 2], c[:, :, 3]
    S1 = EA + EB
    S2 = EA - EB
    S3 = OA + OB
    S4 = OA - OB
    Pl = np.concatenate([S1 + S3, S2 + S4], axis=2)   # normal s2 rows
    Mn = np.concatenate([S1 - S3, S2 - S4], axis=2)   # reflected s2 rows
    n = a.shape[0]
    out = np.empty((n, IMG, IMG), dtype=np.float32)
    out[:, :, 0:256] = _to_s1(Pl).transpose(0, 2, 1)
    out[:, :, 256:512] = _to_s1(Mn).transpose(0, 2, 1)[:, :, ::-1]
    return out


def kernel(x, masks):
    from concourse.bass_utils import run_bass_kernel_spmd

    x = np.ascontiguousarray(np.asarray(x), dtype=np.float32)
    B, C, Hh, W = x.shape
    n = B * C
    per = n // NCORES

    # host folds: rows (xa +- xr) and columns (c' vs 511-c') both commute
    # with the device's M1 row-contraction.
    xs = x.reshape(n, Hh, W)
    xa = xs[:, 0:H, :].reshape(n, 2, P, W).transpose(0, 2, 1, 3)      # [n,P,2,W]
    xr = xs[:, ::-1, :][:, 0:H, :].reshape(n, 2, P, W).transpose(0, 2, 1, 3)
    xp = xa + xr
    xm = xa - xr
    xpf = xp[:, :, :, ::-1]
    xmf = xm[:, :, :, ::-1]
    xq = np.empty((n, P, 4, 2, H), dtype=np.float16)
    xq[:, :, 0] = (xp[:, :, :, 0:H] + xpf[:, :, :, 0:H]).astype(np.float16)  # pp
    xq[:, :, 1] = (xm[:, :, :, 0:H] + xmf[:, :, :, 0:H]).astype(np.float16)  # mp
    xq[:, :, 2] = (xp[:, :, :, 0:H] - xpf[:, :, :, 0:H]).astype(np.float16)  # pm
    xq[:, :, 3] = (xm[:, :, :, 0:H] - xmf[:, :, :, 0:H]).astype(np.float16)  # mm
    consts = _consts()

    in_maps = [
        {"xq": xq[c * per : (c + 1) * per], **consts}
        for c in range(NCORES)
    ]

    nc = _get_prog(per)
    res = run_bass_kernel_spmd(
        nc, in_maps, list(range(NCORES)), trace=TRACE, **TRACE_KWARGS
    )
    global LAST_RESULTS
    LAST_RESULTS = res

    raw = np.concatenate([res.results[c]["out"] for c in range(NCORES)], axis=0)
    outs = {}
    for mi, nm in enumerate(("lh", "hl", "hh")):
        asm = _assemble_lh if nm == "lh" else _assemble_ab
        outs[nm] = asm(np.ascontiguousarray(raw[:, :, :, mi, :])).reshape(B, C, Hh, W)
    LL = x.copy()
    return (LL, outs["lh"], outs["hl"], outs["hh"])
